# revision 17
# baseline (speedup 1.0000x reference)
"""TRN2 Bass kernel for nn_Cvx_KnapsackNet (MLP + 200-iter ADMM projection QP).

Math: the reference ADMM iteration collapses algebraically. With
P' = (I - A^T (A A^T)^{-1} A)/(1+rho), c = b @ (A A^T)^{-1} A, and
state q_k = x_k + u_{k-1}:
    t_k     = w + |q_k|          (t_0 = w)
    x_k     = t_k @ P' + c
    q_{k+1} = x_k + min(q_k, 0)
c is folded into the matmul via an extra "ones" row (row 1030 of the
padded state is held at 1; row 1030 of P' holds c).

Acceleration: the first NPR iterations run *over-relaxed* ADMM with
alpha=2 (Peaceman-Rachford). For alpha=2 the relaxed update
    q' = q + alpha*(x - relu(q))
collapses to q' = 2x - |q|, so with Pt = 2*P' (and 2c in the bias row)
the whole update is one tensor-tensor subtract against the matmul
output. PR roughly halves the iterations needed; NFIN plain ADMM
iterations follow to settle the active set (the plain iterations feed
the matmul t/2 -- using half-scaled |q| and w -- so the same doubled
Pt matrix yields the unscaled x). PR roughly halves the iterations:
4 PR + 2 plain iterations replace the ~16 plain iterations needed to
reach the bf16 fixed-point floor.

Everything runs in bf16 on the PE (fp32 PSUM accumulation): bf16
matmuls are 4x faster than fp32 and halve the dominant W2 HBM stream.
Measured end-to-end error ~4.8e-3 vs the 2e-2 target.

Schedule: HBM-bound MLP, tensor-bound ADMM.
- W2 host-packed partition-major; 5 chunks, each loaded as 2 DMAs so
  compute trails the stream by half a chunk; triple buffered.
- Cost-layer (W3) matmuls interleave into layer-2's DMA shadow,
  accumulating into SBUF so PSUM stays within 8 banks.
- P' (bf16, j-major) streams in 3 DMAs anchored under the last W2
  chunk's compute, just in time for ADMM iteration 0.
- ADMM elementwise per tile: TT subtract (PR) / fp32 STT (plain),
  ScalarE Abs -> bf16, all-bf16 TT add (2x DVE mode).

Sharding: pure data parallel, batch 1024 -> 128 rows per NeuronCore.
On-chip layout is transposed ([n2p=1152 rows, 128 batch cols], 9 tiles
of 128 partitions) so the matmul contraction runs over partitions.
"""
import sys
sys.path.insert(0, '/opt/trn_rl_repo')
import os
import numpy as np

B, C, H, R, K = 1024, 32, 3200, 500, 30
RHO = 1.0
N1 = K + R              # 530
N2 = R + K + R          # 1030
N2P = 1152              # 9 * 128
NT = N2P // 128         # 9 state tiles
BIAS_ROW = N2           # 1030
NCORES = 8
BL = B // NCORES        # 128 batch rows per core
HT = H // 128           # 25 hidden tiles
NPR = int(os.environ.get("KNAP_PR", "4"))     # Peaceman-Rachford iters
NFIN = int(os.environ.get("KNAP_FIN", "2"))   # plain ADMM finishers
MC_W = 5                # m-tiles per W2 chunk
N_MC = HT // MC_W       # 5 chunks
CT = 512 // 128         # 4 cost tiles (500 padded to 512)

_CACHE = {}


def _host_precompute(W1, b1, W2, b2, W3, b3, weights_mat, capacities):
    """float64 host math -> packed bf16/fp32 device constants."""
    import ml_dtypes
    bf = ml_dtypes.bfloat16
    wm = weights_mat.astype(np.float64)
    cap = capacities.astype(np.float64)
    A = np.zeros((N1, N2), np.float64)
    A[:K, :R] = wm
    A[:K, R:R + K] = np.eye(K)
    A[K:, :R] = np.eye(R)
    A[K:, R + K:] = np.eye(R)
    b = np.concatenate([cap, np.ones(R)])
    M = np.linalg.inv(A @ A.T)
    P = (np.eye(N2) - A.T @ M @ A) / (1.0 + RHO)
    c = b @ M @ A
    Pbig = np.zeros((N2P, N2P), np.float32)
    Pbig[:N2, :N2] = 2.0 * P.astype(np.float32)      # Pt = 2 P'
    Pbig[BIAS_ROW, :N2] = 2.0 * c.astype(np.float32)
    # j-major blocked: PbigPM[p, (j*NT+k)*128 + f] = Pbig[k*128+p, j*128+f]
    PbigPM = np.ascontiguousarray(
        Pbig.reshape(NT, 128, NT, 128).transpose(1, 2, 0, 3).reshape(128, NT * NT * 128))
    PbigBF = PbigPM.astype(bf)

    W3p = np.zeros((512, H), np.float32)
    W3p[:R] = W3
    # w3PM[p, k*512 + f] = W3p.T[k*128+p, f]
    w3PM = np.ascontiguousarray(
        W3p.T.reshape(HT, 128, 512).transpose(1, 0, 2).reshape(128, HT * 512)).astype(bf)

    b1R = np.ascontiguousarray(b1.reshape(HT, 128).T)       # [128, 25]
    b2R = np.ascontiguousarray(b2.reshape(HT, 128).T)       # [128, 25]
    b3p = np.zeros(512, np.float32)
    b3p[:R] = b3
    b3R = np.ascontiguousarray(b3p.reshape(CT, 128).T)      # [128, 4]
    b3Rh = 0.5 * b3R                                        # [128, 4]
    # padding tiles 4..8 of w (zeros; bias-row 1030 -> tile 8, partition 6 = 1)
    wpad = np.zeros((128, (NT - CT) * 128), np.float32)
    wpad[BIAS_ROW - 8 * 128, (8 - CT) * 128:(9 - CT) * 128] = 1.0

    small = np.concatenate([b1R, b2R, b3R, b3Rh, wpad], axis=1).astype(np.float32)
    # w2PM[p, (mc*HT + k)*(MC_W*128) + f] = W2.T[k*128+p, mc*MC_W*128+f]
    W2T = np.ascontiguousarray(W2.T)                        # [3200, 3200] (in, out)
    w2PM = np.ascontiguousarray(
        W2T.reshape(HT, 128, N_MC, MC_W * 128).transpose(1, 2, 0, 3)
           .reshape(128, H * H // 128)).astype(bf)          # [128, 80000]
    W1T = np.ascontiguousarray(W1.T).astype(bf)             # [32, 3200]
    return small, PbigBF, w3PM, W1T, w2PM


def _build_nc():
    import concourse.bacc as bacc
    import concourse.mybir as mybir
    from concourse import tile
    from concourse.tile_rust import add_dep_helper

    f32 = mybir.dt.float32
    bf16 = mybir.dt.bfloat16
    SMALL_W = HT + HT + CT + CT + (NT - CT) * 128
    OFF_B1 = 0
    OFF_B2 = OFF_B1 + HT
    OFF_B3 = OFF_B2 + HT
    OFF_B3H = OFF_B3 + CT
    OFF_WP = OFF_B3H + CT
    MCW = MC_W * 128        # 640 cols per W2 chunk
    W2CH = HT * MCW         # 16000 elems per partition per chunk
    W2SPLIT = [0, 9 * MCW, 17 * MCW, W2CH]   # chunk DMA split in thirds
    PBF_W = NT * NT * 128   # 10368
    PBF_CH = [(0, 3), (3, 3), (6, 3)]   # j-ranges per pbf DMA
    TOTAL = NPR + NFIN
    assert NPR >= 1 and NFIN >= 1

    nc = bacc.Bacc("TRN2", target_bir_lowering=False, debug=False, num_devices=NCORES)
    small_d = nc.dram_tensor("small_d", [128, SMALL_W], f32, kind="ExternalInput").ap()
    pbf_d = nc.dram_tensor("pbf_d", [128, PBF_W], bf16, kind="ExternalInput").ap()
    w3_d = nc.dram_tensor("w3_d", [128, HT * 512], bf16, kind="ExternalInput").ap()
    dw_d = nc.dram_tensor("dw_d", [C, BL + H], bf16, kind="ExternalInput").ap()
    w2_d = nc.dram_tensor("w2_d", [128, N_MC * W2CH], bf16, kind="ExternalInput").ap()
    out_d = nc.dram_tensor("out_d", [128, N2P], f32, kind="ExternalOutput").ap()

    Act = mybir.ActivationFunctionType
    Alu = mybir.AluOpType

    with tile.TileContext(nc) as tc:
        with tc.tile_pool(name="sb", bufs=1) as sb, \
             tc.tile_pool(name="wst", bufs=3) as wst, \
             tc.tile_pool(name="mlp", bufs=1) as mlp, \
             tc.tile_pool(name="ps", bufs=8, space="PSUM") as pspool:
            dw = mlp.tile([C, BL + H], bf16)
            nc.sync.dma_start(out=dw[:], in_=dw_d[:])
            sm = sb.tile([128, SMALL_W], f32)
            nc.sync.dma_start(out=sm[:], in_=small_d[:])
            w3sb = sb.tile([128, HT * 512], bf16)
            nc.sync.dma_start(out=w3sb[:], in_=w3_d[:])
            pbf = sb.tile([128, PBF_W], bf16)

            b1R = sm[:, OFF_B1:OFF_B1 + HT]
            b2R = sm[:, OFF_B2:OFF_B2 + HT]
            b3R = sm[:, OFF_B3:OFF_B3 + CT]
            b3Rh = sm[:, OFF_B3H:OFF_B3H + CT]
            dT = dw[:, 0:BL]
            w1T = dw[:, BL:BL + H]

            h1 = mlp.tile([128, HT * 128], bf16)  # h1T tiles: [p, m*128+b]
            h2 = mlp.tile([128, HT * 128], bf16)
            w_acc = mlp.tile([128, 512], f32)     # cost-layer SBUF accumulator
            wb_sb = sb.tile([128, N2P], bf16)     # w      (PR iterations)
            wh_sb = sb.tile([128, N2P], bf16)     # w / 2  (plain iterations)
            q_sb = sb.tile([128, N2P], f32)
            a_sb = sb.tile([128, N2P], bf16)
            tb_bufs = [sb.tile([128, N2P], bf16, name=f"tb{i}") for i in range(3)]
            out_sb = sb.tile([128, N2P], f32)

            nc.vector.memset(a_sb[:], 0.0)
            # pad tiles 4..8 of w (zeros + bias-one row): full and half copies
            nc.vector.tensor_copy(wb_sb[:, CT * 128:],
                                  sm[:, OFF_WP:OFF_WP + (NT - CT) * 128])
            nc.scalar.activation(wh_sb[:, CT * 128:],
                                 sm[:, OFF_WP:OFF_WP + (NT - CT) * 128],
                                 Act.Copy, scale=0.5)

            # ---- MLP layer 1: h1T[m] = prelu(W1T[:,m].T @ dT + b1, 0.1) ----
            for m in range(HT):
                ps_t = pspool.tile([128, 128], f32, tag="ps", name="ps_t")
                nc.tensor.matmul(ps_t[:], w1T[:, m * 128:(m + 1) * 128], dT,
                                 start=True, stop=True)
                nc.scalar.activation(h1[:, m * 128:(m + 1) * 128], ps_t[:],
                                     Act.Prelu, bias=b1R[:, m:m + 1], alpha=0.1)

            # ---- MLP layer 2 + interleaved cost layer ----
            chunk_mark = {}

            def emit_cost(mc):
                ps_c = [pspool.tile([128, 128], f32, tag="ps", name="ps_t")
                        for _ in range(CT)]
                for ki in range(MC_W):
                    k = mc * MC_W + ki
                    for m in range(CT):
                        nc.tensor.matmul(ps_c[m][:],
                                         w3sb[:, k * 512 + m * 128:
                                                 k * 512 + (m + 1) * 128],
                                         h2[:, k * 128:(k + 1) * 128],
                                         start=(ki == 0), stop=(ki == MC_W - 1))
                for m in range(CT):
                    mm = slice(m * 128, (m + 1) * 128)
                    if mc == 0:
                        nc.scalar.activation(w_acc[:, mm], ps_c[m][:], Act.Copy)
                    else:
                        nc.vector.tensor_tensor(out=w_acc[:, mm], in0=w_acc[:, mm],
                                                in1=ps_c[m][:], op=Alu.add)
                if mc == N_MC - 1:
                    for m in range(CT):
                        mm = slice(m * 128, (m + 1) * 128)
                        nc.scalar.activation(wb_sb[:, mm], w_acc[:, mm],
                                             Act.Identity, bias=b3R[:, m:m + 1])
                    for m in range(CT):
                        mm = slice(m * 128, (m + 1) * 128)
                        nc.scalar.activation(wh_sb[:, mm], w_acc[:, mm],
                                             Act.Identity, bias=b3Rh[:, m:m + 1],
                                             scale=0.5)

            for mc in range(N_MC):
                w2blk = wst.tile([128, W2CH], bf16, name="w2blk")
                for s0, s1 in zip(W2SPLIT[:-1], W2SPLIT[1:]):
                    nc.sync.dma_start(out=w2blk[:, s0:s1],
                                      in_=w2_d[:, mc * W2CH + s0:mc * W2CH + s1])
                if mc == N_MC - 1:
                    # P' queues right behind the last W2 chunk on the DMA
                    # engines; program order keeps the stream gap-free.
                    for j0, jn in PBF_CH:
                        nc.sync.dma_start(
                            out=pbf[:, j0 * NT * 128:(j0 + jn) * NT * 128],
                            in_=pbf_d[:, j0 * NT * 128:(j0 + jn) * NT * 128])
                if mc >= 1:
                    # cost matmuls for the previous chunk run in this chunk's
                    # DMA-wait gap and free their PSUM banks early
                    emit_cost(mc - 1)
                ps_list = [pspool.tile([128, 128], f32, tag="ps", name="ps_t")
                           for _ in range(MC_W)]
                last_mc = (mc == N_MC - 1)
                # For the last chunk, the final DMA third runs mi-grouped so
                # each h2 tile (and its cost matmuls) completes as early as
                # possible, shortening the serial tail into ADMM iter 0.
                KSPLIT = 17 if last_mc else HT
                for k in range(KSPLIT):
                    for mi in range(MC_W):
                        mm = nc.tensor.matmul(ps_list[mi][:],
                                         w2blk[:, k * MCW + mi * 128:
                                                  k * MCW + (mi + 1) * 128],
                                         h1[:, k * 128:(k + 1) * 128],
                                         start=(k == 0), stop=(k == HT - 1))
                        if k == 0 and mi == 0:
                            chunk_mark[mc] = mm.ins
                if not last_mc:
                    for mi in range(MC_W):
                        m = mc * MC_W + mi
                        nc.scalar.activation(h2[:, m * 128:(m + 1) * 128],
                                             ps_list[mi][:], Act.Prelu,
                                             bias=b2R[:, m:m + 1], alpha=0.1)
            # ---- last chunk tail: mi-grouped third + pipelined cost ----
            mc = N_MC - 1
            ps_c = [pspool.tile([128, 128], f32, tag="ps", name="ps_t")
                    for _ in range(CT)]

            def cost_piece(ki):
                k = mc * MC_W + ki
                for m in range(CT):
                    nc.tensor.matmul(ps_c[m][:],
                                     w3sb[:, k * 512 + m * 128:
                                             k * 512 + (m + 1) * 128],
                                     h2[:, k * 128:(k + 1) * 128],
                                     start=(ki == 0), stop=(ki == MC_W - 1))

            for mi in range(MC_W):
                for k in range(17, HT):
                    nc.tensor.matmul(ps_list[mi][:],
                                     w2blk[:, k * MCW + mi * 128:
                                              k * MCW + (mi + 1) * 128],
                                     h1[:, k * 128:(k + 1) * 128],
                                     start=False, stop=(k == HT - 1))
                m = mc * MC_W + mi
                nc.scalar.activation(h2[:, m * 128:(m + 1) * 128], ps_list[mi][:],
                                     Act.Prelu, bias=b2R[:, m:m + 1], alpha=0.1)
                if mi >= 1:
                    cost_piece(mi - 1)
            cost_piece(MC_W - 1)
            for m in range(CT):
                mm = slice(m * 128, (m + 1) * 128)
                nc.vector.tensor_tensor(out=w_acc[:, mm], in0=w_acc[:, mm],
                                        in1=ps_c[m][:], op=Alu.add)
            for m in range(CT):
                mm = slice(m * 128, (m + 1) * 128)
                nc.scalar.activation(wb_sb[:, mm], w_acc[:, mm],
                                     Act.Identity, bias=b3R[:, m:m + 1])
            for m in range(CT):
                mm = slice(m * 128, (m + 1) * 128)
                nc.scalar.activation(wh_sb[:, mm], w_acc[:, mm],
                                     Act.Identity, bias=b3Rh[:, m:m + 1],
                                     scale=0.5)

            # ---- ADMM: NPR Peaceman-Rachford iters + NFIN plain iters ----
            for it in range(TOTAL):
                last = (it == TOTAL - 1)
                pr = it < NPR
                half_out = (it >= NPR - 1)      # feed t/2 to plain iterations
                cur = wb_sb if it == 0 else tb_bufs[(it - 1) % 3]
                for j in range(NT):
                    ps_t = pspool.tile([128, 128], f32, tag="ps", name="ps_t")
                    for k in range(NT):
                        nc.tensor.matmul(ps_t[:],
                                         pbf[:, (j * NT + k) * 128:(j * NT + k + 1) * 128],
                                         cur[:, k * 128:(k + 1) * 128],
                                         start=(k == 0), stop=(k == NT - 1))
                    jj = slice(j * 128, (j + 1) * 128)
                    if last:
                        nc.scalar.activation(out_sb[:, jj], ps_t[:], Act.Copy)
                        if j % 3 == 2:
                            oo = slice((j - 2) * 128, (j + 1) * 128)
                            nc.sync.dma_start(out=out_d[:, oo], in_=out_sb[:, oo])
                        continue
                    if pr:
                        # q' = 2x - |q|  (Pt already holds the factor 2)
                        nc.vector.tensor_tensor(out=q_sb[:, jj], in0=ps_t[:],
                                                in1=a_sb[:, jj], op=Alu.subtract)
                    else:
                        # q' = min(q, 0) + x
                        nc.vector.scalar_tensor_tensor(
                            out=q_sb[:, jj], in0=q_sb[:, jj], scalar=0.0,
                            in1=ps_t[:], op0=Alu.min, op1=Alu.add)
                    nc.scalar.activation(a_sb[:, jj], q_sb[:, jj], Act.Abs,
                                         scale=0.5 if half_out else 1.0)
                    nc.vector.tensor_tensor(out=tb_bufs[it % 3][:, jj],
                                            in0=a_sb[:, jj],
                                            in1=(wh_sb if half_out else wb_sb)[:, jj],
                                            op=Alu.add)

    nc.compile()
    return nc


def kernel(d, W1, b1, W2, b2, W3, b3, weights_mat, capacities):
    import ml_dtypes
    from concourse.bass_utils import run_bass_kernel_spmd

    d = np.asarray(d, np.float32)
    small, PbigBF, w3PM, W1T, w2PM = _host_precompute(
        np.asarray(W1, np.float32), np.asarray(b1, np.float32),
        np.asarray(W2, np.float32), np.asarray(b2, np.float32),
        np.asarray(W3, np.float32), np.asarray(b3, np.float32),
        np.asarray(weights_mat, np.float32), np.asarray(capacities, np.float32))

    if "nc" not in _CACHE:
        _CACHE["nc"] = _build_nc()
    nc = _CACHE["nc"]

    in_maps = []
    for i in range(NCORES):
        dTc = np.ascontiguousarray(d[i * BL:(i + 1) * BL].T)      # [32, 128]
        dwc = np.concatenate([dTc.astype(ml_dtypes.bfloat16), W1T], axis=1)
        in_maps.append({"small_d": small, "pbf_d": PbigBF,
                        "w3_d": w3PM, "dw_d": dwc, "w2_d": w2PM})

    trace = bool(int(os.environ.get("KNAP_TRACE", "0")))
    res = run_bass_kernel_spmd(nc, in_maps, core_ids=list(range(NCORES)),
                               trace=trace)
    if trace:
        _CACHE["exec_time_ns"] = res.exec_time_ns
        _CACHE["trace"] = res.instructions_and_trace

    out = np.empty((B, N2), np.float32)
    for i in range(NCORES):
        arr = res.results[i]["out_d"]                              # [128, 1152]
        xc = arr.reshape(128, NT, 128).transpose(2, 1, 0).reshape(BL, N2P)
        out[i * BL:(i + 1) * BL] = xc[:, :N2]
    return out
